# revision 1
# baseline (speedup 1.0000x reference)
"""DGCNN forward kernel for 8 Trainium2 NeuronCores.

Contract: kernel(**inputs) takes the FULL inputs of the reference
(x:(4,3,8192), w1..w5) and returns the FULL output (4,512,8192) fp32.

Sharding: data-parallel over batch B=4 x query-halves -> 8 cores.
Core c = 2*b + h computes queries [h*4096,(h+1)*4096) of batch item b
against all 8192 candidates of batch item b. No cross-core comm.

Per-core pipeline (query tiles of 128):
  PE    : score matmuls s_qj = 2*x_q.x_j - |x_j|^2   (fp32, K=4)
  ACT   : PSUM->SBUF copies of the (128,8192) score block
  DVE   : max8 (top-8 values) + max_index (top-8 indices) -> exact top-5
  SWDGE : indirect DMA gather of neighbor coords (128B padded rows)
  PE    : per-k fp32 transpose of [nbr;ctr] + conv1..conv5 (fp32)
  ACT   : relu epilogues
  DVE   : max-pool over K=5
  DMA   : output store
"""

import sys

if '/opt/trn_rl_repo' not in sys.path:
    sys.path.insert(0, '/opt/trn_rl_repo')

import numpy as np

import concourse.bass as bass
import concourse.tile as tile
from concourse import bacc, mybir
from concourse.bass_utils import run_bass_kernel_spmd

F32 = mybir.dt.float32
F32R = mybir.dt.float32r
U32 = mybir.dt.uint32
AF = mybir.ActivationFunctionType
ALU = mybir.AluOpType

B = 4
N = 8192          # points per batch element (candidates)
NQ = 4096         # queries per core
P = 128           # queries per tile
SG = 4            # tiles per supergroup (conv5 free dim = SG*128 = 512)
KNN = 5


def _build_program(n=N, nq=NQ, sgsz=SG, num_devices=8, stop_after=None):
    NT_ = nq // P
    NSG_ = NT_ // sgsz
    nc = bacc.Bacc("TRN2", target_bir_lowering=False, debug=False,
                   num_devices=num_devices)

    d_xt32 = nc.dram_tensor("xt32", [n, 32], F32, kind="ExternalInput").ap()
    d_srhs = nc.dram_tensor("srhs", [4, n], F32, kind="ExternalInput").ap()
    d_xq4 = nc.dram_tensor("xq4", [4, nq], F32, kind="ExternalInput").ap()
    d_w1t = nc.dram_tensor("w1t", [6, 64], F32, kind="ExternalInput").ap()
    d_w2t = nc.dram_tensor("w2t", [64, 64], F32, kind="ExternalInput").ap()
    d_w3t = nc.dram_tensor("w3t", [128, 128], F32, kind="ExternalInput").ap()
    d_w4t = nc.dram_tensor("w4t", [128, 256], F32, kind="ExternalInput").ap()
    d_w5r = nc.dram_tensor("w5r", [128, 2048], F32, kind="ExternalInput").ap()
    d_idn = nc.dram_tensor("idn", [128, 128], F32, kind="ExternalInput").ap()
    d_out = nc.dram_tensor("out", [512, nq], F32, kind="ExternalOutput").ap()

    with tile.TileContext(nc) as tc:
        with tc.tile_pool(name="consts", bufs=1) as consts, \
             tc.tile_pool(name="scores", bufs=2) as scores_pool, \
             tc.tile_pool(name="small", bufs=2) as small, \
             tc.tile_pool(name="acts", bufs=2) as acts, \
             tc.tile_pool(name="cats", bufs=2) as cats, \
             tc.tile_pool(name="ps_score", bufs=2, space="PSUM") as ps_score, \
             tc.tile_pool(name="ps_work", bufs=2, space="PSUM") as ps_work:

            srhs = consts.tile([4, n], F32)
            nc.sync.dma_start(srhs[:], d_srhs[:])
            xq4 = consts.tile([4, nq], F32)
            nc.sync.dma_start(xq4[:], d_xq4[:])
            w1t = consts.tile([6, 64], F32)
            nc.sync.dma_start(w1t[:], d_w1t[:])
            w2t = consts.tile([64, 64], F32)
            nc.sync.dma_start(w2t[:], d_w2t[:])
            w3t = consts.tile([128, 128], F32)
            nc.sync.dma_start(w3t[:], d_w3t[:])
            w4t = consts.tile([128, 256], F32)
            nc.sync.dma_start(w4t[:], d_w4t[:])
            w5r = consts.tile([128, 2048], F32)
            nc.sync.dma_start(w5r[:], d_w5r[:])
            idn = consts.tile([128, 128], F32)
            nc.sync.dma_start(idn[:], d_idn[:])
            w3r = consts.tile([128, 128], F32R)
            nc.vector.tensor_copy(w3r[:], w3t[:])
            w4r = consts.tile([128, 256], F32R)
            nc.vector.tensor_copy(w4r[:], w4t[:])
            w5rr = consts.tile([128, 2048], F32R)
            nc.vector.tensor_copy(w5rr[:], w5r[:])

            out_view = d_out.rearrange("(o p) q -> p o q", o=4)
            _early = ("scores", "topk", "gather", "ti", "conv1")

            for sg in range(NSG_):
                if stop_after in _early:
                    cat12 = cat3 = cat4a = cat4b = None
                else:
                    cat12 = cats.tile([128, sgsz * P], F32R, tag="cat12")
                    cat3 = cats.tile([128, sgsz * P], F32R, tag="cat3")
                    cat4a = cats.tile([128, sgsz * P], F32R, tag="cat4a")
                    cat4b = cats.tile([128, sgsz * P], F32R, tag="cat4b")

                for ti in range(sgsz):
                    t = sg * sgsz + ti
                    q0 = t * P

                    # ---- scores: s (128 q, n cand) ----
                    sc = scores_pool.tile([P, n], F32, tag="sc")
                    lhsq = xq4[:, q0:q0 + P]
                    for cc in range(n // 1024):
                        psc = ps_score.tile([P, 1024], F32, tag="psc")
                        c0 = cc * 1024
                        nc.tensor.matmul(psc[:, 0:512], lhsT=lhsq,
                                         rhs=srhs[:, c0:c0 + 512],
                                         start=True, stop=True)
                        nc.tensor.matmul(psc[:, 512:1024], lhsT=lhsq,
                                         rhs=srhs[:, c0 + 512:c0 + 1024],
                                         start=True, stop=True)
                        nc.scalar.activation(sc[:, c0:c0 + 1024], psc[:], AF.Copy)

                    # ---- top-5 (exact, fp32) ----
                    m8 = small.tile([P, 8], F32, tag="m8")
                    nc.vector.max(out=m8[:], in_=sc[:])
                    i8 = small.tile([P, 8], U32, tag="i8")
                    nc.vector.max_index(out=i8[:], in_max=m8[:], in_values=sc[:])

                    if stop_after == "scores":
                        dbg = small.tile([P, P], F32, tag="dbg")
                        nc.vector.tensor_copy(dbg[:], sc[:, 0:P])
                        nc.sync.dma_start(d_out[0:P, t * P:(t + 1) * P], dbg[:])
                        continue
                    if stop_after == "topk":
                        dbg = small.tile([P, P], F32, tag="dbg")
                        nc.vector.memset(dbg[:], 0.0)
                        nc.vector.tensor_copy(dbg[:, 0:8], m8[:])
                        nc.vector.tensor_copy(dbg[:, 8:16], i8[:])
                        nc.sync.dma_start(d_out[0:P, t * P:(t + 1) * P], dbg[:])
                        continue

                    # ---- gather neighbor coords: g[q, k, :] = xt32[idx[q,k]] ----
                    # one offset per partition per DMA (multi-offset indirect
                    # DMA scrambles on HW)
                    g = small.tile([P, KNN, 32], F32, tag="g")
                    for k in range(KNN):
                        nc.gpsimd.indirect_dma_start(
                            out=g[:, k, :],
                            out_offset=None,
                            in_=d_xt32[:],
                            in_offset=bass.IndirectOffsetOnAxis(
                                ap=i8[:, k:k + 1], axis=0),
                        )

                    if stop_after == "gather":
                        dbg = small.tile([P, P], F32, tag="dbg")
                        nc.vector.tensor_copy(
                            dbg[:], g[:].rearrange("p k j -> p (k j)")[:, 0:P])
                        nc.sync.dma_start(d_out[0:P, t * P:(t + 1) * P], dbg[:])
                        continue

                    # ---- assemble TI[q, k, 0:6] = [nbr_k(3), ctr(3)] ----
                    # ctr = gathered top-1 row (self) broadcast over k.
                    tin = small.tile([P, KNN, 6], F32, tag="tin")
                    nc.vector.tensor_copy(tin[:, :, 0:3], g[:, :, 0:3])
                    nc.vector.tensor_copy(
                        tin[:, :, 3:6], g[:, 0:1, 0:3].to_broadcast([P, KNN, 3]))

                    if stop_after == "ti":
                        dbg = small.tile([P, P], F32, tag="dbg")
                        nc.vector.memset(dbg[:], 0.0)
                        nc.vector.tensor_copy(
                            dbg[:, 0:30], tin[:].rearrange("p k j -> p (k j)"))
                        nc.sync.dma_start(d_out[0:P, t * P:(t + 1) * P], dbg[:])
                        continue

                    # ---- per-k transpose (128,6)->(6,128), conv1 K=6 ----
                    ps_tp = ps_work.tile([8, KNN * P], F32, tag="work")
                    for k in range(KNN):
                        nc.tensor.transpose(ps_tp[0:6, k * P:(k + 1) * P],
                                            tin[:, k, :], idn[:])
                    tps = small.tile([8, KNN * P], F32, tag="tps")
                    nc.scalar.activation(tps[0:6, :], ps_tp[0:6, :], AF.Copy)

                    ps_h1 = ps_work.tile([64, KNN * P], F32, tag="work")
                    for k in range(KNN):
                        nc.tensor.matmul(ps_h1[:, k * P:(k + 1) * P],
                                         lhsT=w1t[:],
                                         rhs=tps[0:6, k * P:(k + 1) * P],
                                         start=True, stop=True)
                    h12 = acts.tile([128, KNN, P], F32R, tag="h12")
                    h1 = h12[0:64]
                    nc.scalar.activation(
                        h12[:].rearrange("c k q -> c (k q)")[0:64, :],
                        ps_h1[:], AF.Relu)

                    if stop_after == "conv1":
                        dbg = small.tile([P, P], F32, tag="dbg")
                        nc.vector.memset(dbg[:], 0.0)
                        nc.vector.tensor_copy(
                            dbg[0:64, :],
                            h12[0:64].rearrange("c k q -> c (k q)")[:, 0:P].bitcast(F32))
                        nc.sync.dma_start(d_out[0:P, t * P:(t + 1) * P], dbg[:])
                        continue

                    # ---- conv2 (output placed at PSUM partitions 64:128) ----
                    ps_c2 = ps_work.tile([128, KNN * P], F32, tag="work")
                    h1f = h12[0:64].rearrange("c k q -> c (k q)").bitcast(F32)
                    nc.tensor.matmul(ps_c2[64:128, 0:512], lhsT=w2t[:],
                                     rhs=h1f[:, 0:512], start=True, stop=True)
                    nc.tensor.matmul(ps_c2[64:128, 512:640], lhsT=w2t[:],
                                     rhs=h1f[:, 512:640], start=True, stop=True)
                    nc.scalar.activation(
                        h12[:].rearrange("c k q -> c (k q)")[64:128, :],
                        ps_c2[64:128, :], AF.Relu)
                    h2 = h12

                    # ---- conv3 (weights live at partitions 64:128) ----
                    ps_c3 = ps_work.tile([128, KNN * P], F32, tag="work")
                    h2f = h2[:].rearrange("c k q -> c (k q)")
                    nc.tensor.matmul(ps_c3[:, 0:512], lhsT=w3r[64:128, :],
                                     rhs=h2f[64:128, 0:512], start=True, stop=True)
                    nc.tensor.matmul(ps_c3[:, 512:640], lhsT=w3r[64:128, :],
                                     rhs=h2f[64:128, 512:640], start=True, stop=True)
                    h3 = acts.tile([128, KNN, P], F32R, tag="h3")
                    nc.scalar.activation(h3[:].rearrange("c k q -> c (k q)"),
                                         ps_c3[:], AF.Relu)

                    # ---- conv4 (256 out channels = two 128 halves) ----
                    h3f = h3[:].rearrange("c k q -> c (k q)")
                    h4 = []
                    for half in range(2):
                        ps_c4 = ps_work.tile([128, KNN * P], F32, tag="work")
                        w4sl = w4r[:, half * 128:(half + 1) * 128]
                        nc.tensor.matmul(ps_c4[:, 0:512], lhsT=w4sl,
                                         rhs=h3f[:, 0:512], start=True, stop=True)
                        nc.tensor.matmul(ps_c4[:, 512:640], lhsT=w4sl,
                                         rhs=h3f[:, 512:640], start=True, stop=True)
                        h4t = acts.tile([128, KNN, P], F32R, tag=f"h4{half}")
                        nc.scalar.activation(h4t[:].rearrange("c k q -> c (k q)"),
                                             ps_c4[:], AF.Relu)
                        h4.append(h4t)

                    # ---- max over K=5 into the supergroup cat tiles ----
                    csl = slice(ti * P, (ti + 1) * P)
                    nc.vector.tensor_reduce(cat12[:, csl],
                                            h12[:].rearrange("c k q -> c q k"),
                                            axis=mybir.AxisListType.X, op=ALU.max)
                    nc.vector.tensor_reduce(cat3[:, csl],
                                            h3[:].rearrange("c k q -> c q k"),
                                            axis=mybir.AxisListType.X, op=ALU.max)
                    nc.vector.tensor_reduce(cat4a[:, csl],
                                            h4[0][:].rearrange("c k q -> c q k"),
                                            axis=mybir.AxisListType.X, op=ALU.max)
                    nc.vector.tensor_reduce(cat4b[:, csl],
                                            h4[1][:].rearrange("c k q -> c q k"),
                                            axis=mybir.AxisListType.X, op=ALU.max)

                if stop_after in _early:
                    continue
                if stop_after == "pools":
                    dbg2 = small.tile([P, sgsz * P], F32, tag="dbg2")
                    nc.vector.tensor_copy(dbg2[:], cat3[:])
                    nc.sync.dma_start(
                        d_out[0:P, sg * sgsz * P:(sg + 1) * sgsz * P], dbg2[:])
                    continue

                # ---- conv5 over the supergroup: K=512 as 4 chunks of 128 ----
                kchunk_rhs = (cat12, cat3, cat4a, cat4b)
                for o in range(4):
                    ps_c5 = ps_work.tile([128, sgsz * P], F32, tag="work")
                    for kk in range(4):
                        nc.tensor.matmul(
                            ps_c5[:],
                            lhsT=w5rr[:, kk * 512 + o * 128:kk * 512 + (o + 1) * 128],
                            rhs=kchunk_rhs[kk][:],
                            start=(kk == 0), stop=(kk == 3))
                    ostage = small.tile([128, sgsz * P], F32, tag="ostage")
                    nc.scalar.activation(ostage[:], ps_c5[:], AF.Relu)
                    nc.sync.dma_start(
                        out_view[:, o, sg * sgsz * P:(sg + 1) * sgsz * P],
                        ostage[:])

    nc.compile()
    return nc


_PROGRAM = None


def _get_program():
    global _PROGRAM
    if _PROGRAM is None:
        _PROGRAM = _build_program()
    return _PROGRAM


def _host_inputs(xb, h, w1, w2, w3, w4, w5, n=N, nq=NQ):
    """Per-core input map for batch element xb (3,n), query slice h."""
    xb = np.ascontiguousarray(xb, dtype=np.float32)
    sq = (xb * xb).sum(axis=0, dtype=np.float32)

    xt32 = np.zeros((n, 32), np.float32)
    xt32[:, 0:3] = xb.T

    srhs = np.empty((4, n), np.float32)
    srhs[0:3] = 2.0 * xb
    srhs[3] = -sq

    q = slice(h * nq, (h + 1) * nq)
    xq4 = np.empty((4, nq), np.float32)
    xq4[0:3] = xb[:, q]
    xq4[3] = 1.0

    w3t = np.zeros((128, 128), np.float32)
    w3t[64:128, :] = w3.T

    w5t = w5.T.astype(np.float32)  # (512 in, 512 out)
    w5r = np.zeros((128, 2048), np.float32)
    for kk in range(4):
        for o in range(4):
            w5r[:, kk * 512 + o * 128:kk * 512 + (o + 1) * 128] = \
                w5t[kk * 128:(kk + 1) * 128, o * 128:(o + 1) * 128]

    return {
        "xt32": xt32,
        "srhs": srhs,
        "xq4": xq4,
        "w1t": np.ascontiguousarray(w1.T, np.float32),
        "w2t": np.ascontiguousarray(w2.T, np.float32),
        "w3t": w3t,
        "w4t": np.ascontiguousarray(w4.T, np.float32),
        "w5r": w5r,
        "idn": np.eye(128, dtype=np.float32),
    }


def kernel(x, w1, w2, w3, w4, w5, _trace=False, _trace_kwargs=None):
    x = np.asarray(x, np.float32)
    w1 = np.asarray(w1, np.float32)
    w2 = np.asarray(w2, np.float32)
    w3 = np.asarray(w3, np.float32)
    w4 = np.asarray(w4, np.float32)
    w5 = np.asarray(w5, np.float32)
    assert x.shape == (B, 3, N), x.shape

    nc = _get_program()
    in_maps = []
    for b in range(B):
        for h in range(2):
            in_maps.append(_host_inputs(x[b], h, w1, w2, w3, w4, w5))

    kw = {}
    if _trace:
        kw = dict(trace=True, **(_trace_kwargs or {}))
    res = run_bass_kernel_spmd(nc, in_maps, list(range(8)), **kw)

    out = np.empty((B, 512, N), np.float32)
    for b in range(B):
        out[b, :, 0:NQ] = res.results[2 * b]["out"]
        out[b, :, NQ:N] = res.results[2 * b + 1]["out"]
    if _trace:
        return out, res
    return out



# revision 2
# speedup vs baseline: 1.0091x; 1.0091x over previous
"""DGCNN forward kernel for 8 Trainium2 NeuronCores — v2.

Contract: kernel(**inputs) takes FULL inputs (x:(4,3,8192), w1..w5),
returns FULL output (4,512,8192) fp32.

Sharding: core c = 2*b + h handles batch b, query half h (4096 queries
vs all 8192 candidates). No cross-core comm.

Per-core pipeline (query tiles of 128):
  PE    : scores s = 2x.y - |y|^2 via bf16 hi/lo split (14 contraction
          rows, 1 cyc/row), fp32 PSUM
  ACT   : biased fp16 copy PSUM->SBUF: sc16 = fp16(s - |x_q|^2)
  DVE   : contiguous-group max tree 8192 -> 1024 group maxima (groups
          of 8), Max8 + MaxIndex on the 1024
  SWDGE : gather the top-5 groups' 8 xt32 rows each (1KB contiguous)
  DVE   : exact rescore of the 40 finalists from coords, self-masked,
          (value|index) mantissa-packed Max8 -> top-4 neighbor indices
  SWDGE : gather the 4 neighbor xt32 rows; k0 = self from host table
  PE    : per-k transpose + conv1..conv5 (fp32r, 2x320-col splits)
  ACT   : relu epilogues;  DVE: max-pool over K=5
"""

import sys

if '/opt/trn_rl_repo' not in sys.path:
    sys.path.insert(0, '/opt/trn_rl_repo')

import numpy as np

import concourse.bass as bass
import concourse.tile as tile
from concourse import bacc, mybir
from concourse.bass_utils import run_bass_kernel_spmd

F32 = mybir.dt.float32
F32R = mybir.dt.float32r
F16 = mybir.dt.float16
BF16 = mybir.dt.bfloat16
U32 = mybir.dt.uint32
AF = mybir.ActivationFunctionType
ALU = mybir.AluOpType

B = 4
N = 8192
NQ = 4096
P = 128
SG = 4
KNN = 5
NGRP = N // 8        # 1024 groups of 8 candidates
TOPG = 5             # groups gathered per query


def _build_program(n=N, nq=NQ, sgsz=SG, num_devices=8, stop_after=None):
    NT_ = nq // P
    NSG_ = NT_ // sgsz
    nc = bacc.Bacc("TRN2", target_bir_lowering=False, debug=False,
                   num_devices=num_devices)

    d_xt32 = nc.dram_tensor("xt32", [n, 32], F32, kind="ExternalInput").ap()
    d_xq14 = nc.dram_tensor("xq14", [14, nq], BF16, kind="ExternalInput").ap()
    d_yrhs = nc.dram_tensor("yrhs", [14, n], BF16, kind="ExternalInput").ap()
    d_negc = nc.dram_tensor("negc", [P, NT_], F32, kind="ExternalInput").ap()
    d_q2x = nc.dram_tensor("q2x", [P, NT_], F32, kind="ExternalInput").ap()
    d_q2y = nc.dram_tensor("q2y", [P, NT_], F32, kind="ExternalInput").ap()
    d_q2z = nc.dram_tensor("q2z", [P, NT_], F32, kind="ExternalInput").ap()
    d_qxyz = nc.dram_tensor("qxyz", [P, 3 * NT_], F32, kind="ExternalInput").ap()
    d_selfi = nc.dram_tensor("selfi", [P, NT_], U32, kind="ExternalInput").ap()
    d_mtab = nc.dram_tensor("mtab", [P, 8], U32, kind="ExternalInput").ap()
    d_cmask = nc.dram_tensor("cmask", [P, 1], U32, kind="ExternalInput").ap()
    d_cneg = nc.dram_tensor("cneg", [P, 1], F32, kind="ExternalInput").ap()
    d_w1t = nc.dram_tensor("w1t", [6, 64], F32, kind="ExternalInput").ap()
    d_w2t = nc.dram_tensor("w2t", [64, 64], F32, kind="ExternalInput").ap()
    d_w3t = nc.dram_tensor("w3t", [128, 128], F32, kind="ExternalInput").ap()
    d_w4t = nc.dram_tensor("w4t", [128, 256], F32, kind="ExternalInput").ap()
    d_w5r = nc.dram_tensor("w5r", [128, 2048], F32, kind="ExternalInput").ap()
    d_idn = nc.dram_tensor("idn", [128, 128], F32, kind="ExternalInput").ap()
    d_out = nc.dram_tensor("out", [512, nq], F32, kind="ExternalOutput").ap()

    # group view of xt32: row g = 8 consecutive point-rows (1KB)
    d_xtg = d_xt32.rearrange("(g e) c -> g (e c)", e=8)

    with tile.TileContext(nc) as tc:
        with tc.tile_pool(name="consts", bufs=1) as consts, \
             tc.tile_pool(name="scores", bufs=2) as scores_pool, \
             tc.tile_pool(name="trees", bufs=3) as trees, \
             tc.tile_pool(name="small", bufs=3) as small, \
             tc.tile_pool(name="acts", bufs=2) as acts, \
             tc.tile_pool(name="cats", bufs=2) as cats, \
             tc.tile_pool(name="ps_score", bufs=2, space="PSUM") as ps_score, \
             tc.tile_pool(name="ps_work", bufs=2, space="PSUM") as ps_work:

            xq14 = consts.tile([14, nq], BF16)
            nc.sync.dma_start(xq14[:], d_xq14[:])
            yrhs = consts.tile([14, n], BF16)
            nc.sync.dma_start(yrhs[:], d_yrhs[:])
            negc = consts.tile([P, NT_], F32)
            nc.sync.dma_start(negc[:], d_negc[:])
            q2x = consts.tile([P, NT_], F32)
            nc.sync.dma_start(q2x[:], d_q2x[:])
            q2y = consts.tile([P, NT_], F32)
            nc.sync.dma_start(q2y[:], d_q2y[:])
            q2z = consts.tile([P, NT_], F32)
            nc.sync.dma_start(q2z[:], d_q2z[:])
            qxyz = consts.tile([P, 3 * NT_], F32)
            nc.sync.dma_start(qxyz[:], d_qxyz[:])
            selfi = consts.tile([P, NT_], U32)
            nc.sync.dma_start(selfi[:], d_selfi[:])
            mtab = consts.tile([P, 8], U32)
            nc.sync.dma_start(mtab[:], d_mtab[:])
            cmask = consts.tile([P, 1], U32)
            nc.sync.dma_start(cmask[:], d_cmask[:])
            cneg = consts.tile([P, 1], F32)
            nc.sync.dma_start(cneg[:], d_cneg[:])
            w1t = consts.tile([6, 64], F32)
            nc.sync.dma_start(w1t[:], d_w1t[:])
            w2t = consts.tile([64, 64], F32)
            nc.sync.dma_start(w2t[:], d_w2t[:])
            w3t = consts.tile([128, 128], F32)
            nc.sync.dma_start(w3t[:], d_w3t[:])
            w4t = consts.tile([128, 256], F32)
            nc.sync.dma_start(w4t[:], d_w4t[:])
            w5r = consts.tile([128, 2048], F32)
            nc.sync.dma_start(w5r[:], d_w5r[:])
            idn = consts.tile([128, 128], F32)
            nc.sync.dma_start(idn[:], d_idn[:])
            w1r = consts.tile([6, 64], F32R)
            nc.vector.tensor_copy(w1r[:], w1t[:])
            w2r = consts.tile([64, 64], F32R)
            nc.vector.tensor_copy(w2r[:], w2t[:])
            w3r = consts.tile([128, 128], F32R)
            nc.vector.tensor_copy(w3r[:], w3t[:])
            w4r = consts.tile([128, 256], F32R)
            nc.vector.tensor_copy(w4r[:], w4t[:])
            w5rr = consts.tile([128, 2048], F32R)
            nc.vector.tensor_copy(w5rr[:], w5r[:])

            out_view = d_out.rearrange("(o p) q -> p o q", o=4)
            _early = ("scores16", "groups", "idx", "gather", "ti", "conv1")

            for sg in range(NSG_):
                if stop_after in _early:
                    cat12 = cat3 = cat4a = cat4b = None
                else:
                    cat12 = cats.tile([128, sgsz * P], F32R, tag="cat12")
                    cat3 = cats.tile([128, sgsz * P], F32R, tag="cat3")
                    cat4a = cats.tile([128, sgsz * P], F32R, tag="cat4a")
                    cat4b = cats.tile([128, sgsz * P], F32R, tag="cat4b")

                for ti in range(sgsz):
                    t = sg * sgsz + ti
                    q0 = t * P

                    # ---- scores (bf16 split matmul) + biased fp16 copy ----
                    sc16 = scores_pool.tile([P, n], F16, tag="sc16")
                    lhsq = xq14[:, q0:q0 + P]
                    for cc in range(n // 1024):
                        psc = ps_score.tile([P, 1024], F32, tag="psc")
                        c0 = cc * 1024
                        nc.tensor.matmul(psc[:, 0:512], lhsT=lhsq,
                                         rhs=yrhs[:, c0:c0 + 512],
                                         start=True, stop=True)
                        nc.tensor.matmul(psc[:, 512:1024], lhsT=lhsq,
                                         rhs=yrhs[:, c0 + 512:c0 + 1024],
                                         start=True, stop=True)
                        if cc == 7:
                            # rebalance: DVE does one of the 8 chunk copies
                            nc.vector.scalar_tensor_tensor(
                                out=sc16[:, c0:c0 + 1024], in0=psc[:],
                                scalar=negc[:, t:t + 1],
                                in1=cneg[:].to_broadcast([P, 1024]),
                                op0=ALU.add, op1=ALU.max)
                        else:
                            nc.scalar.activation(sc16[:, c0:c0 + 1024], psc[:],
                                                 AF.Identity,
                                                 bias=negc[:, t:t + 1])

                    if stop_after == "scores16":
                        dbg = small.tile([P, P], F32, tag="dbg")
                        nc.vector.tensor_copy(dbg[:], sc16[:, 0:P])
                        nc.sync.dma_start(d_out[0:P, t * P:(t + 1) * P], dbg[:])
                        continue

                    # ---- group max tree: 8192 -> 1024 (groups of 8) ----
                    scv = sc16[:].rearrange("p (g e) -> p g e", e=8)
                    t1 = trees.tile([P, NGRP, 4], F16, tag="t1")
                    nc.vector.tensor_tensor(t1[:], scv[:, :, 0:4],
                                            scv[:, :, 4:8], ALU.max)
                    t2 = trees.tile([P, NGRP, 2], F16, tag="t2")
                    nc.vector.tensor_tensor(t2[:], t1[:, :, 0:2],
                                            t1[:, :, 2:4], ALU.max)
                    red = trees.tile([P, NGRP], F16, tag="red")
                    nc.vector.tensor_tensor(red[:], t2[:, :, 0],
                                            t2[:, :, 1], ALU.max)

                    # ---- top-8 group maxima + positions ----
                    m8 = small.tile([P, 8], F16, tag="m8")
                    nc.vector.max(out=m8[:], in_=red[:])
                    j8 = small.tile([P, 8], U32, tag="j8")
                    nc.vector.max_index(out=j8[:], in_max=m8[:],
                                        in_values=red[:])

                    if stop_after == "groups":
                        dbg = small.tile([P, P], F32, tag="dbg")
                        nc.vector.memset(dbg[:], 0.0)
                        nc.vector.tensor_copy(dbg[:, 0:8], m8[:])
                        nc.vector.tensor_copy(dbg[:, 8:16], j8[:])
                        nc.sync.dma_start(d_out[0:P, t * P:(t + 1) * P], dbg[:])
                        continue

                    # ---- gather top-5 groups' member rows (1KB each) ----
                    grp = small.tile([P, TOPG, 8, 32], F32, tag="grp")
                    for i in range(TOPG):
                        nc.gpsimd.indirect_dma_start(
                            out=grp[:, i, :, :].rearrange("p e c -> p (e c)"),
                            out_offset=None,
                            in_=d_xtg[:],
                            in_offset=bass.IndirectOffsetOnAxis(
                                ap=j8[:, i:i + 1], axis=0),
                        )

                    # ---- exact rescore of the 40 finalists ----
                    r1 = small.tile([P, TOPG, 8], F32, tag="r1")
                    nc.vector.scalar_tensor_tensor(
                        out=r1[:], in0=grp[:, :, :, 0], scalar=q2x[:, t:t + 1],
                        in1=grp[:, :, :, 3], op0=ALU.mult, op1=ALU.subtract)
                    r2 = small.tile([P, TOPG, 8], F32, tag="r2")
                    nc.vector.scalar_tensor_tensor(
                        out=r2[:], in0=grp[:, :, :, 1], scalar=q2y[:, t:t + 1],
                        in1=r1[:], op0=ALU.mult, op1=ALU.add)
                    r3 = small.tile([P, TOPG, 8], F32, tag="r3")
                    nc.vector.scalar_tensor_tensor(
                        out=r3[:], in0=grp[:, :, :, 2], scalar=q2z[:, t:t + 1],
                        in1=r2[:], op0=ALU.mult, op1=ALU.add)
                    # biased fp16 (clears low mantissa) then back to fp32
                    r16 = small.tile([P, TOPG * 8], F16, tag="r16")
                    nc.scalar.activation(
                        r16[:], r3[:].rearrange("p i e -> p (i e)"),
                        AF.Identity, bias=negc[:, t:t + 1])
                    rf = small.tile([P, TOPG * 8], F32, tag="rf")
                    nc.vector.tensor_copy(rf[:], r16[:])

                    # global candidate index of each finalist: 8*j + m
                    gidx = small.tile([P, TOPG, 8], U32, tag="gidx")
                    nc.vector.scalar_tensor_tensor(
                        out=gidx[:],
                        in0=j8[:, 0:TOPG].rearrange("p (i u) -> p i u", u=1)
                            .to_broadcast([P, TOPG, 8]),
                        scalar=8.0,
                        in1=mtab[:].rearrange("p (u e) -> p u e", u=1)
                            .to_broadcast([P, TOPG, 8]),
                        op0=ALU.mult, op1=ALU.add)
                    gidxf = gidx[:].rearrange("p i e -> p (i e)")

                    # mask self to -2^17, then pack value|index in mantissa
                    eq = small.tile([P, TOPG * 8], U32, tag="eq")
                    nc.vector.tensor_tensor(
                        eq[:], gidxf,
                        selfi[:, t:t + 1].to_broadcast([P, TOPG * 8]),
                        ALU.is_equal)
                    masked = small.tile([P, TOPG * 8], F32, tag="masked")
                    nc.vector.scalar_tensor_tensor(
                        out=masked[:], in0=eq[:], scalar=-131072.0,
                        in1=rf[:], op0=ALU.mult, op1=ALU.add)
                    packed = small.tile([P, TOPG * 8], U32, tag="packed")
                    nc.vector.tensor_tensor(packed[:], masked[:].bitcast(U32),
                                            gidxf, ALU.bitwise_or)

                    p8 = small.tile([P, 8], F32, tag="p8")
                    nc.vector.max(out=p8[:], in_=packed[:].bitcast(F32))
                    i4 = small.tile([P, 4], U32, tag="i4")
                    nc.vector.tensor_tensor(
                        i4[:], p8[:, 0:4].bitcast(U32),
                        cmask[:].to_broadcast([P, 4]), ALU.bitwise_and)

                    if stop_after == "idx":
                        dbg = small.tile([P, P], F32, tag="dbg")
                        nc.vector.memset(dbg[:], 0.0)
                        nc.vector.tensor_copy(dbg[:, 0:4], i4[:])
                        nc.vector.tensor_copy(dbg[:, 4:12], j8[:])
                        nc.vector.tensor_copy(dbg[:, 12:52], rf[:])
                        nc.sync.dma_start(d_out[0:P, t * P:(t + 1) * P], dbg[:])
                        continue

                    # ---- gather the 4 neighbor coord rows ----
                    g4 = small.tile([P, 4, 32], F32, tag="g4")
                    for k in range(4):
                        nc.gpsimd.indirect_dma_start(
                            out=g4[:, k, :],
                            out_offset=None,
                            in_=d_xt32[:],
                            in_offset=bass.IndirectOffsetOnAxis(
                                ap=i4[:, k:k + 1], axis=0),
                        )

                    if stop_after == "gather":
                        dbg = small.tile([P, P], F32, tag="dbg")
                        nc.vector.memset(dbg[:], 0.0)
                        nc.vector.tensor_copy(
                            dbg[:, 0:P],
                            g4[:].rearrange("p k j -> p (k j)")[:, 0:P])
                        nc.sync.dma_start(d_out[0:P, t * P:(t + 1) * P], dbg[:])
                        continue

                    # ---- assemble TI[q, k, 0:6] = [nbr_k(3), ctr(3)] ----
                    qsl = qxyz[:, 3 * t:3 * t + 3]
                    tin = small.tile([P, KNN, 6], F32, tag="tin")
                    nc.vector.tensor_copy(tin[:, 1:5, 0:3], g4[:, :, 0:3])
                    nc.vector.tensor_copy(
                        tin[:, :, 3:6],
                        qsl.rearrange("p (u c) -> p u c", u=1)
                           .to_broadcast([P, KNN, 3]))
                    nc.vector.tensor_copy(tin[:, 0, 0:3], qsl)

                    if stop_after == "ti":
                        dbg = small.tile([P, P], F32, tag="dbg")
                        nc.vector.memset(dbg[:], 0.0)
                        nc.vector.tensor_copy(
                            dbg[:, 0:30], tin[:].rearrange("p k j -> p (k j)"))
                        nc.sync.dma_start(d_out[0:P, t * P:(t + 1) * P], dbg[:])
                        continue

                    # ---- per-k transpose (128,6)->(6,128), conv1 ----
                    ps_tp = ps_work.tile([8, KNN * P], F32, tag="work")
                    for k in range(KNN):
                        nc.tensor.transpose(ps_tp[0:6, k * P:(k + 1) * P],
                                            tin[:, k, :], idn[:])
                    tps = small.tile([8, KNN * P], F32R, tag="tps")
                    nc.scalar.activation(tps[0:6, :], ps_tp[0:6, :], AF.Copy)

                    ps_h1 = ps_work.tile([64, KNN * P], F32, tag="work")
                    nc.tensor.matmul(ps_h1[:, 0:512], lhsT=w1r[:],
                                     rhs=tps[0:6, 0:512], start=True, stop=True)
                    nc.tensor.matmul(ps_h1[:, 512:640], lhsT=w1r[:],
                                     rhs=tps[0:6, 512:640], start=True, stop=True)
                    h12 = acts.tile([128, KNN, P], F32R, tag="h12")
                    nc.scalar.activation(
                        h12[:].rearrange("c k q -> c (k q)")[0:64, :],
                        ps_h1[:], AF.Relu)

                    if stop_after == "conv1":
                        dbg = small.tile([P, P], F32, tag="dbg")
                        nc.vector.memset(dbg[:], 0.0)
                        nc.vector.tensor_copy(
                            dbg[0:64, :],
                            h12[0:64].rearrange("c k q -> c (k q)")[:, 0:P].bitcast(F32))
                        nc.sync.dma_start(d_out[0:P, t * P:(t + 1) * P], dbg[:])
                        continue

                    # ---- conv2 (into PSUM partitions 64:128) ----
                    ps_c2 = ps_work.tile([128, KNN * P], F32, tag="work")
                    h1f = h12[0:64].rearrange("c k q -> c (k q)").bitcast(F32)
                    nc.tensor.matmul(ps_c2[64:128, 0:512], lhsT=w2t[:],
                                     rhs=h1f[:, 0:512], start=True, stop=True)
                    nc.tensor.matmul(ps_c2[64:128, 512:640], lhsT=w2t[:],
                                     rhs=h1f[:, 512:640], start=True, stop=True)
                    nc.scalar.activation(
                        h12[:].rearrange("c k q -> c (k q)")[64:128, :],
                        ps_c2[64:128, :], AF.Relu)
                    h2 = h12

                    # ---- conv3 (weights at partitions 64:128) ----
                    ps_c3 = ps_work.tile([128, KNN * P], F32, tag="work")
                    h2f = h2[:].rearrange("c k q -> c (k q)")
                    nc.tensor.matmul(ps_c3[:, 0:512], lhsT=w3r[64:128, :],
                                     rhs=h2f[64:128, 0:512], start=True, stop=True)
                    nc.tensor.matmul(ps_c3[:, 512:640], lhsT=w3r[64:128, :],
                                     rhs=h2f[64:128, 512:640], start=True, stop=True)
                    h3 = acts.tile([128, KNN, P], F32R, tag="h3")
                    nc.scalar.activation(h3[:].rearrange("c k q -> c (k q)"),
                                         ps_c3[:], AF.Relu)

                    # ---- conv4 (256 out = two 128 halves) ----
                    h3f = h3[:].rearrange("c k q -> c (k q)")
                    h4 = []
                    for half in range(2):
                        ps_c4 = ps_work.tile([128, KNN * P], F32, tag="work")
                        w4sl = w4r[:, half * 128:(half + 1) * 128]
                        nc.tensor.matmul(ps_c4[:, 0:512], lhsT=w4sl,
                                         rhs=h3f[:, 0:512], start=True, stop=True)
                        nc.tensor.matmul(ps_c4[:, 512:640], lhsT=w4sl,
                                         rhs=h3f[:, 512:640], start=True, stop=True)
                        h4t = acts.tile([128, KNN, P], F32R, tag=f"h4{half}")
                        nc.scalar.activation(h4t[:].rearrange("c k q -> c (k q)"),
                                             ps_c4v if False else ps_c4[:], AF.Relu)
                        h4.append(h4t)

                    # ---- max over K=5 into supergroup cat tiles ----
                    csl = slice(ti * P, (ti + 1) * P)
                    nc.vector.tensor_reduce(cat12[:, csl],
                                            h12[:].rearrange("c k q -> c q k"),
                                            axis=mybir.AxisListType.X, op=ALU.max)
                    nc.vector.tensor_reduce(cat3[:, csl],
                                            h3[:].rearrange("c k q -> c q k"),
                                            axis=mybir.AxisListType.X, op=ALU.max)
                    nc.vector.tensor_reduce(cat4a[:, csl],
                                            h4[0][:].rearrange("c k q -> c q k"),
                                            axis=mybir.AxisListType.X, op=ALU.max)
                    nc.vector.tensor_reduce(cat4b[:, csl],
                                            h4[1][:].rearrange("c k q -> c q k"),
                                            axis=mybir.AxisListType.X, op=ALU.max)

                if stop_after in _early:
                    continue
                if stop_after == "pools":
                    dbg2 = small.tile([P, sgsz * P], F32, tag="dbg2")
                    nc.vector.tensor_copy(dbg2[:], cat3[:])
                    nc.sync.dma_start(
                        d_out[0:P, sg * sgsz * P:(sg + 1) * sgsz * P], dbg2[:])
                    continue

                # ---- conv5: K=512 as 4 chunks of 128 ----
                kchunk_rhs = (cat12, cat3, cat4a, cat4b)
                for o in range(4):
                    ps_c5 = ps_work.tile([128, sgsz * P], F32, tag="work")
                    for kk in range(4):
                        nc.tensor.matmul(
                            ps_c5[:],
                            lhsT=w5rr[:, kk * 512 + o * 128:kk * 512 + (o + 1) * 128],
                            rhs=kchunk_rhs[kk][:],
                            start=(kk == 0), stop=(kk == 3))
                    ostage = small.tile([128, sgsz * P], F32, tag="ostage")
                    nc.scalar.activation(ostage[:], ps_c5[:], AF.Relu)
                    nc.sync.dma_start(
                        out_view[:, o, sg * sgsz * P:(sg + 1) * sgsz * P],
                        ostage[:])

    nc.compile()
    return nc


_PROGRAM = None


def _get_program():
    global _PROGRAM
    if _PROGRAM is None:
        _PROGRAM = _build_program()
    return _PROGRAM


def _bf16(a):
    a32 = np.ascontiguousarray(a, np.float32)
    u = a32.view(np.uint32)
    r = ((u >> 16) + ((u >> 15) & 1)).astype(np.uint32) << 16
    return r.view(np.float32)


def _host_inputs(xb, h, w1, w2, w3, w4, w5, n=N, nq=NQ):
    """Per-core input map for batch element xb (3,n), query half h."""
    import ml_dtypes
    xb = np.ascontiguousarray(xb, dtype=np.float32)
    sq = (xb * xb).sum(axis=0, dtype=np.float32)
    NT_ = nq // P

    xt32 = np.zeros((n, 32), np.float32)
    xt32[:, 0:3] = xb.T
    xt32[:, 3] = sq

    q = slice(h * nq, (h + 1) * nq)
    a = 2.0 * xb[:, q]                      # (3, nq)
    ah = _bf16(a); al = _bf16(a - ah)
    xq14 = np.ones((14, nq), np.float32)
    xq14[0:3] = ah; xq14[3:6] = al
    xq14[6:9] = ah; xq14[9:12] = al
    # rows 12, 13 stay 1.0

    yh = _bf16(xb); yl = _bf16(xb - yh)
    sqh = _bf16(sq); sql = _bf16(sq - sqh)
    yrhs = np.empty((14, n), np.float32)
    yrhs[0:3] = yh; yrhs[3:6] = yh
    yrhs[6:9] = yl; yrhs[9:12] = yl
    yrhs[12] = -sqh; yrhs[13] = -sql

    sqq = sq[q].reshape(NT_, P).T           # (128, NT)
    negc = np.ascontiguousarray(-sqq, np.float32)
    xqv = xb[:, q]                           # (3, nq)
    q2 = (2.0 * xqv).reshape(3, NT_, P)
    q2x = np.ascontiguousarray(q2[0].T, np.float32)
    q2y = np.ascontiguousarray(q2[1].T, np.float32)
    q2z = np.ascontiguousarray(q2[2].T, np.float32)
    qxyz = np.ascontiguousarray(
        xqv.reshape(3, NT_, P).transpose(2, 1, 0).reshape(P, 3 * NT_),
        np.float32)
    selfi = np.ascontiguousarray(
        (h * nq + np.arange(nq, dtype=np.uint32)).reshape(NT_, P).T)
    mtab = np.broadcast_to(np.arange(8, dtype=np.uint32), (P, 8)).copy()
    cmask = np.full((P, 1), 0x1FFF, np.uint32)
    cneg = np.full((P, 1), -131072.0, np.float32)

    w3t = np.zeros((128, 128), np.float32)
    w3t[64:128, :] = w3.T

    w5t = w5.T.astype(np.float32)
    w5r = np.zeros((128, 2048), np.float32)
    for kk in range(4):
        for o in range(4):
            w5r[:, kk * 512 + o * 128:kk * 512 + (o + 1) * 128] = \
                w5t[kk * 128:(kk + 1) * 128, o * 128:(o + 1) * 128]

    return {
        "xt32": xt32,
        "xq14": xq14.astype(ml_dtypes.bfloat16),
        "yrhs": yrhs.astype(ml_dtypes.bfloat16),
        "negc": negc, "q2x": q2x, "q2y": q2y, "q2z": q2z,
        "qxyz": qxyz, "selfi": selfi, "mtab": mtab,
        "cmask": cmask, "cneg": cneg,
        "w1t": np.ascontiguousarray(w1.T, np.float32),
        "w2t": np.ascontiguousarray(w2.T, np.float32),
        "w3t": w3t,
        "w4t": np.ascontiguousarray(w4.T, np.float32),
        "w5r": w5r,
        "idn": np.eye(128, dtype=np.float32),
    }


def kernel(x, w1, w2, w3, w4, w5, _trace=False, _trace_kwargs=None):
    x = np.asarray(x, np.float32)
    w1 = np.asarray(w1, np.float32)
    w2 = np.asarray(w2, np.float32)
    w3 = np.asarray(w3, np.float32)
    w4 = np.asarray(w4, np.float32)
    w5 = np.asarray(w5, np.float32)
    assert x.shape == (B, 3, N), x.shape

    nc = _get_program()
    in_maps = []
    for b in range(B):
        for h in range(2):
            in_maps.append(_host_inputs(x[b], h, w1, w2, w3, w4, w5))

    kw = {}
    if _trace:
        kw = dict(trace=True, **(_trace_kwargs or {}))
    res = run_bass_kernel_spmd(nc, in_maps, list(range(8)), **kw)

    out = np.empty((B, 512, N), np.float32)
    for b in range(B):
        out[b, :, 0:NQ] = res.results[2 * b]["out"]
        out[b, :, NQ:N] = res.results[2 * b + 1]["out"]
    if _trace:
        return out, res
    return out
